# revision 1
# baseline (speedup 1.0000x reference)
"""Dual-pixel depth-merge (forward splat) kernel for Trainium2, 8 NeuronCores.

Math: for integer pixel grid x, the reference computes pos = fl(x +- depth)
(f32-rounded), x0 = floor(pos), f = pos - x0. Define the per-view fractional
offsets
    v_l[i] = fl(i + depth[i]) - i   (exact f32 subtraction, in [0, 8])
    v_r[i] = i - fl(i - depth[i])   (exact f32 subtraction, in [0, 8])
Then each view's splat is a 9-tap shifted weighted sum with hat weights
    Wl_d = relu(1 - |v_l - d|),  Wr_d = relu(1 - |v_r - d|),  d = 0..8:
    count_l[j] = sum_d Wl_d[j-d]      acc_l[c,j] = sum_d (Wl_d*img_c)[j-d]
    count_r[j] = sum_d Wr_d[j+d]      acc_r[c,j] = sum_d (Wr_d*img_c)[j+d]
    left = acc_l / max(count_l, eps)  right = acc_r / max(count_r, eps)
This reproduces the reference's weights bit-for-bit (matching its f32
rounding of x+-depth), so count==0 happens exactly where the reference's
does — and there acc==0 too, making the eps-divide equal the reference's
where(count==0, 1, count).

Sharding: pure data parallel over h (the scatter is along w only) — core m
takes h rows [m*128, (m+1)*128) for all batches. No halo, no communication.
"""

import numpy as np

import concourse.bacc as bacc
import concourse.bass as bass
import concourse.mybir as mybir
import concourse.tile as tile
from concourse.bass_utils import run_bass_kernel_spmd

B, C, H, W = 4, 3, 1024, 1024
NCORES = 8
HS = H // NCORES  # 128 h-rows per core
NTAP = 9
F32 = mybir.dt.float32
EPS = 1e-20

_MAX = mybir.AluOpType.max
_ADD = mybir.AluOpType.add
_SUB = mybir.AluOpType.subtract
_RELU = mybir.ActivationFunctionType.Relu
_ABS = mybir.ActivationFunctionType.Abs
_IDENT = mybir.ActivationFunctionType.Identity


def _bcast_c(ap):
    """View a [HS, W] tile as [HS, C, W] by repeating along a step-0 dim."""
    a = ap.ap
    return bass.AP(tensor=ap.tensor, offset=ap.offset, ap=[list(a[0]), [0, C], list(a[1])])


CFG = {
    "io": 2, "w": 4, "v": 2, "t": 2, "p": 3, "acc": 2, "accn": 2,
    # Column split: DVE handles w in [0, spl), GPSIMD handles [spl, W), for
    # products/adds (spl_p) and count sums (spl_c).
    "spl_p": 688,
    "spl_a": 656,
    "spl_c": 688,
}


def build_program(skip: frozenset = frozenset(), cfg: dict | None = None) -> bass.Bass:
    """skip: debug knob — subset of {"counts", "prods", "weights"} to omit
    (produces wrong results; used only for critical-path bisection)."""
    cfg = {**CFG, **(cfg or {})}
    nc = bacc.Bacc()
    image = nc.dram_tensor("image", [B, C, HS, W], F32, kind="ExternalInput")
    depth = nc.dram_tensor("depth", [B, HS, W], F32, kind="ExternalInput")
    left = nc.dram_tensor("left", [B, C, HS, W], F32, kind="ExternalOutput")
    right = nc.dram_tensor("right", [B, C, HS, W], F32, kind="ExternalOutput")

    with tile.TileContext(nc) as tc:
        with (
            tc.tile_pool(name="consts", bufs=1) as c_pool,
            tc.tile_pool(name="io", bufs=cfg["io"]) as io_pool,
            tc.tile_pool(name="wts", bufs=cfg["w"]) as w_pool,
            tc.tile_pool(name="voff", bufs=cfg["v"]) as v_pool,
            tc.tile_pool(name="tmp", bufs=cfg["t"]) as t_pool,
            tc.tile_pool(name="prod", bufs=cfg["p"]) as p_pool,
            tc.tile_pool(name="accs", bufs=cfg["acc"]) as acc_pool,
            tc.tile_pool(name="accn", bufs=cfg["accn"]) as accn_pool,
        ):
            # Per-tap bias constants and the column-index (iota) row.
            negd = c_pool.tile([HS, NTAP], F32, tag="negd")
            for d in range(NTAP):
                nc.vector.memset(negd[:, d : d + 1], -float(d))
            epsb = c_pool.tile([HS, 1], F32, tag="epsb")
            nc.vector.memset(epsb[:], EPS)
            iota_i = t_pool.tile([HS, W], mybir.dt.int32, tag="t")
            nc.gpsimd.iota(iota_i[:], [[1, W]], channel_multiplier=0)
            iota = c_pool.tile([HS, W], F32, tag="iota")
            nc.vector.tensor_copy(iota[:], iota_i[:])

            for b in range(B):
                dep = io_pool.tile([HS, W], F32, tag="dep")
                nc.sync.dma_start(out=dep[:], in_=depth[b])
                img = io_pool.tile([HS, C, W], F32, tag="img")
                nc.sync.dma_start(out=img[:], in_=image[b].transpose([1, 0, 2]))

                # Exact per-view fractional offsets (reproduce reference's
                # f32 rounding of x +- depth; the second subtract is exact).
                vl = v_pool.tile([HS, W], F32, tag="vl")
                vr = v_pool.tile([HS, W], F32, tag="vr")
                s = t_pool.tile([HS, W], F32, tag="s")
                nc.vector.tensor_tensor(s[:], dep[:], iota[:], _ADD)
                nc.vector.tensor_tensor(vl[:], s[:], iota[:], _SUB)
                s2 = t_pool.tile([HS, W], F32, tag="s")
                nc.gpsimd.tensor_tensor(s2[:], iota[:], dep[:], _SUB)
                nc.gpsimd.tensor_tensor(vr[:], iota[:], s2[:], _SUB)

                # Interleave the two views tap-by-tap so DVE/GPSIMD/ACT all
                # stay fed. Work is column-split: DVE takes [0, spl),
                # GPSIMD [spl, W) of every product/add/count op.
                sp = cfg["spl_p"]
                sa = cfg["spl_a"]
                sc = cfg["spl_c"]
                views = (("l", vl), ("r", vr))
                cnt_l = accn_pool.tile([HS, W], F32, tag="cl")
                cnt_r = accn_pool.tile([HS, W], F32, tag="cr")
                acc_l = acc_pool.tile([HS, C, W], F32, tag="al")
                acc_r = acc_pool.tile([HS, C, W], F32, tag="ar")
                cnts = {"l": cnt_l, "r": cnt_r}
                accs = {"l": acc_l, "r": acc_r}
                for d in range(NTAP):
                    for view, v in views:
                        cnt, acc = cnts[view], accs[view]
                        # Tap weight W_d = relu(1 - |v - d|) on the scalar engine.
                        td = t_pool.tile([HS, W], F32, tag="t")
                        nc.scalar.activation(td[:], v[:], _ABS, bias=negd[:, d : d + 1], scale=1.0)
                        wd = w_pool.tile([HS, W], F32, tag="w")
                        nc.scalar.activation(wd[:], td[:], _RELU, bias=1.0, scale=-1.0)
                        if d == 0:
                            nc.scalar.copy(cnt[:], wd[:])
                            nc.vector.tensor_mul(acc[:, :, 0:sp], _bcast_c(wd[:, 0:sp]), img[:, :, 0:sp])
                            nc.gpsimd.tensor_mul(acc[:, :, sp:W], _bcast_c(wd[:, sp:W]), img[:, :, sp:W])
                            continue
                        if "counts" not in skip:
                            if view == "l":
                                nc.vector.tensor_tensor(cnt[:, d:sc], cnt[:, d:sc], wd[:, 0 : sc - d], _ADD)
                                nc.gpsimd.tensor_tensor(cnt[:, sc:W], cnt[:, sc:W], wd[:, sc - d : W - d], _ADD)
                            else:
                                nc.vector.tensor_tensor(cnt[:, 0:sc], cnt[:, 0:sc], wd[:, d : sc + d], _ADD)
                                nc.gpsimd.tensor_tensor(cnt[:, sc : W - d], cnt[:, sc : W - d], wd[:, sc + d : W], _ADD)
                        if "prods" not in skip:
                            pd = p_pool.tile([HS, C, W], F32, tag="p")
                            nc.vector.tensor_mul(pd[:, :, 0:sp], _bcast_c(wd[:, 0:sp]), img[:, :, 0:sp])
                            nc.gpsimd.tensor_mul(pd[:, :, sp:W], _bcast_c(wd[:, sp:W]), img[:, :, sp:W])
                            if view == "l":
                                nc.vector.tensor_add(acc[:, :, d:sa], acc[:, :, d:sa], pd[:, :, 0 : sa - d])
                                nc.gpsimd.tensor_add(acc[:, :, sa:W], acc[:, :, sa:W], pd[:, :, sa - d : W - d])
                            else:
                                nc.vector.tensor_add(acc[:, :, 0:sa], acc[:, :, 0:sa], pd[:, :, d : sa + d])
                                nc.gpsimd.tensor_add(acc[:, :, sa : W - d], acc[:, :, sa : W - d], pd[:, :, sa + d : W])

                # Normalize: out = acc * (1 / max(count, eps)).
                for view, _ in views:
                    cnt, acc = cnts[view], accs[view]
                    # count >= 0 and its smallest nonzero value is ~6e-8, so
                    # count + 1e-20 is bit-identical to max(count, 1e-20) —
                    # and an add-constant runs on the idle scalar engine.
                    rc = accn_pool.tile([HS, W], F32, tag=f"rc{view}")
                    nc.scalar.activation(cnt[:], cnt[:], _IDENT, bias=epsb[:], scale=1.0)
                    nc.vector.reciprocal_approx_fast(out=rc[:], in_=cnt[:])
                    nc.vector.tensor_mul(acc[:, :, 0:sa], acc[:, :, 0:sa], _bcast_c(rc[:, 0:sa]))
                    nc.gpsimd.tensor_mul(acc[:, :, sa:W], acc[:, :, sa:W], _bcast_c(rc[:, sa:W]))

                nc.sync.dma_start(out=left[b].transpose([1, 0, 2]), in_=accs["l"][:])
                nc.sync.dma_start(out=right[b].transpose([1, 0, 2]), in_=accs["r"][:])
    nc.compile()
    return nc


_NC_CACHE = None


def _get_program():
    global _NC_CACHE
    if _NC_CACHE is None:
        _NC_CACHE = build_program()
    return _NC_CACHE


def kernel(image: np.ndarray, depth: np.ndarray):
    image = np.ascontiguousarray(image, dtype=np.float32)
    depth = np.ascontiguousarray(depth, dtype=np.float32)
    assert image.shape == (B, C, H, W) and depth.shape == (B, H, W)

    nc = _get_program()
    in_maps = []
    for m in range(NCORES):
        sl = slice(m * HS, (m + 1) * HS)
        in_maps.append(
            {
                "image": np.ascontiguousarray(image[:, :, sl, :]),
                "depth": np.ascontiguousarray(depth[:, sl, :]),
            }
        )
    # The axon-tunneled devices occasionally come up in a transient
    # unrecoverable/desynced state (e.g. poisoned by a previous failed
    # process) and recover on the next attempt — retry once before giving up.
    try:
        res = run_bass_kernel_spmd(nc, in_maps, core_ids=list(range(NCORES)))
    except Exception:
        import time as _time

        _time.sleep(5.0)
        res = run_bass_kernel_spmd(nc, in_maps, core_ids=list(range(NCORES)))
    left = np.concatenate([r["left"] for r in res.results], axis=2)
    right = np.concatenate([r["right"] for r in res.results], axis=2)
    return left, right



# revision 21
# speedup vs baseline: 1.2489x; 1.2489x over previous
"""Dual-pixel depth-merge (forward splat) kernel for Trainium2, 8 NeuronCores.

Math: for integer pixel grid x, the reference computes pos = fl(x +- depth)
(f32-rounded), x0 = floor(pos), f = pos - x0. Define the per-view fractional
offsets
    v_l[i] = fl(i + depth[i]) - i   (exact f32 subtraction, in [0, 8])
    v_r[i] = i - fl(i - depth[i])   (exact f32 subtraction, in [0, 8])
Each view's splat is a 9-tap shifted weighted sum with hat weights
    W_d = relu(1 - |v - d|), d = 0..8.

The bulk of the pipeline runs in fp16 with weights scaled by S=1024:
w' = relu(1024 - |1024 v - 1024 d|). The scaling keeps every nonzero f32
weight (>= 2^-24) a NORMAL fp16 value (>= 2^-14), so hole positions
(count == 0) match the reference bit-exactly and near-hole pixels stay
relatively accurate; the scale cancels in acc/count.

Layout: per tap a padded tile P_d [128, 2(view), 4(rgb+count), 1032]; view l
data sits at column offset +d, view r at 8-d (its output is read at +8), and
the unused margin columns are zeroed, so every accumulation is a plain
aligned full-tile add. Taps accumulate in TWO independent chains (even taps
into acc_e, odd into acc_o) merged at the end — halving the serial-add
critical path. Each add can be column-split across three resources: DVE
tensor_tensor (fp16 2x), GP tensor_tensor, and gpsimd accumulate-DMA
(CCE add on the DMA engines). View r accumulates NEGATED weights (DVE's
(t-1024) min 0 tensor_scalar form, 2x); the sign cancels in acc/count.

Engine roles: Act = weight stage 1 + view-l stage 2 + eps/rc16; DVE =
view-r stage 2, most products, add slices, reciprocal, final mul; GP = add
slices, margin memsets, SWDGE descriptor gen; DMA = i/o + CCE add slices.
Sharding: pure data parallel over h (the scatter is along w only) — core m
takes h rows [m*128, (m+1)*128). No halo, no communication.
"""

import numpy as np

import concourse.bacc as bacc
import concourse.bass as bass
import concourse.mybir as mybir
import concourse.tile as tile
from concourse.bass_utils import run_bass_kernel_spmd

B, C, H, W = 4, 3, 1024, 1024
NCORES = 8
HS = H // NCORES  # 128 h-rows per core
NTAP = 9
PAD = 8
WP = W + PAD  # padded accumulator width
F16 = mybir.dt.float16
F32 = mybir.dt.float32
SCALE = 1024.0
EPS = 1e-10

_ADD = mybir.AluOpType.add
_SUB = mybir.AluOpType.subtract
_MULT = mybir.AluOpType.mult
_MIN = mybir.AluOpType.min
_BYPASS = mybir.AluOpType.bypass
_RELU = mybir.ActivationFunctionType.Relu
_ABS = mybir.ActivationFunctionType.Abs
_IDENT = mybir.ActivationFunctionType.Identity


def _ap(tile_ap, extra_off, dims):
    """Build an AP on tile_ap's tensor: offset += extra_off (elements),
    dims = [[step, num], ...] appended after the partition dim."""
    a = tile_ap.ap
    return bass.AP(
        tensor=tile_ap.tensor,
        offset=tile_ap.offset + extra_off,
        ap=[list(a[0])] + [list(d) for d in dims],
    )


# The 8 accumulates: E-chain adds taps 2,4,6,8 into acc_e; O-chain adds taps
# 3,5,7 into acc_o; "mg" is the final acc_e += acc_o merge. Each value is a
# column split [dve_cols, gp_cols, dma_cols] summing to WP=1032.
CFG = {
    # Accumulation: 4 independent pair-adds (P1+=P2, P3+=P4, P5+=P6,
    # P7+=P8) then a chain into the root tap-0 tile (R+=S1..S4). Engine per
    # add: 'v' DVE, 'g' GP STT, 'm' gpsimd accumulate-DMA.
    # CCE accumulate-DMA ('m') wedges the real device (NRT_EXEC_UNIT_
    # UNRECOVERABLE) even though it models well, so adds stay on DVE/GP.
    "tree": ["v", {"v": 720, "g": 312}, {"v": 720, "g": 312}, "v",
             "v", {"v": 720, "g": 312}, {"v": 720, "g": 312}, "v"],
    "prod_gp_cols": 0,   # product columns (per view) handled by GP
    "prod_gp_taps": 5,   # taps (d < this) eligible for the GP product split
    "off_gp": 2,         # offset ops on GP (0, 2 or 4)
    "mul_gp_cols": 384,    # final-mul columns (per view) handled by GP
    "msets_gp": True,    # margin memsets on GP (else DVE)
    "cce_single_packet": True,
    "cast_in": True,     # load image via casting gpsimd DMA
    "cast_out": True,    # store outputs via casting gpsimd DMA
    "io_bufs": 2,
    "p_bufs": 5,
    "acc_bufs": 2,
    "out_bufs": 1,
}


def build_program(cfg: dict | None = None) -> bass.Bass:
    cfg = {**CFG, **(cfg or {})}
    nc = bacc.Bacc()
    image = nc.dram_tensor("image", [B, C, HS, W], F32, kind="ExternalInput")
    depth = nc.dram_tensor("depth", [B, HS, W], F32, kind="ExternalInput")
    left = nc.dram_tensor("left", [B, C, HS, W], F32, kind="ExternalOutput")
    right = nc.dram_tensor("right", [B, C, HS, W], F32, kind="ExternalOutput")

    with tile.TileContext(nc) as tc:
        with (
            tc.tile_pool(name="consts", bufs=1) as c_pool,
            tc.tile_pool(name="io", bufs=cfg["io_bufs"]) as io_pool,
            tc.tile_pool(name="voff", bufs=2) as v_pool,
            tc.tile_pool(name="wts", bufs=2) as w_pool,
            tc.tile_pool(name="prod", bufs=cfg["p_bufs"]) as p_pool,
            tc.tile_pool(name="root", bufs=cfg["acc_bufs"]) as r_pool,
            tc.tile_pool(name="norm", bufs=2) as n_pool,
            tc.tile_pool(name="outs", bufs=cfg["out_bufs"]) as o_pool,
        ):
            iota = c_pool.tile([HS, W], F32, tag="iota", name="iota")
            iota_iv = iota[:].bitcast(mybir.dt.int32)
            nc.gpsimd.iota(iota_iv, [[1, W]], channel_multiplier=0)
            nc.vector.tensor_copy(iota[:], iota_iv)
            epsp = c_pool.tile([HS, 1], F32, tag="epsp", name="epsp")
            nc.vector.memset(epsp[:], EPS)
            epsn = c_pool.tile([HS, 1], F32, tag="epsn", name="epsn")
            nc.vector.memset(epsn[:], -EPS)
            # Per-tap stage-1 biases (-1024*d) and the stage-2 bias (+1024).
            negd = c_pool.tile([HS, NTAP], F32, tag="negd", name="negd")
            for d in range(NTAP):
                nc.vector.memset(negd[:, d : d + 1], -SCALE * d)
            b1024 = c_pool.tile([HS, 1], F32, tag="b1024", name="b1024")
            nc.vector.memset(b1024[:], SCALE)

            def full_add(dst, src, spec):
                """dst += src over the whole [HS, 2, 4, WP] tile. spec is an
                engine char ('v' DVE, 'g' GP, 'm' accumulate-DMA) or a dict
                {eng: cols} column-splitting the add across engines."""
                if isinstance(spec, str):
                    spec = {spec: WP}
                a = 0
                for eng, wid in spec.items():
                    if wid == 0:
                        continue
                    daps = _ap(dst[:], a, [[WP, 8], [1, wid]])
                    saps = _ap(src[:], a, [[WP, 8], [1, wid]])
                    if eng == "v":
                        nc.vector.tensor_tensor(daps, daps, saps, _ADD)
                    elif eng == "g":
                        nc.gpsimd.tensor_tensor(daps, daps, saps, _ADD)
                    else:
                        nc.gpsimd.dma_start(out=daps, in_=saps, accum_op=_ADD, single_packet=cfg["cce_single_packet"])
                    a += wid
                    assert a <= WP


            for b in range(B):
                dep = io_pool.tile([HS, W], F32, tag="dep", name="dep")
                nc.sync.dma_start(out=dep[:], in_=depth[b])
                if cfg["cast_in"]:
                    img = io_pool.tile([HS, C, W], F16, tag="img", name="img")
                    nc.gpsimd.dma_start(out=img[:], in_=image[b].transpose([1, 0, 2]))
                else:
                    img32 = io_pool.tile([HS, C, W], F32, tag="img32", name="img32")
                    nc.sync.dma_start(out=img32[:], in_=image[b].transpose([1, 0, 2]))
                    img = io_pool.tile([HS, C, W], F16, tag="img", name="img")
                    nc.vector.tensor_copy(img[:], img32[:])

                # v offsets, f32-exact. vcat[:,0,:] = vl, vcat[:,1,:] = vr.
                vcat = v_pool.tile([HS, 2, W], F32, tag="vcat", name="vcat")
                s_l = v_pool.tile([HS, W], F32, tag="sl", name="s_l", bufs=1)
                s_r = v_pool.tile([HS, W], F32, tag="sr", name="s_r", bufs=1)
                og = cfg["off_gp"]
                if og >= 2:
                    nc.gpsimd.tensor_tensor(s_l[:], dep[:], iota[:], _ADD)
                    nc.gpsimd.tensor_tensor(vcat[:, 0, :], s_l[:], iota[:], _SUB)
                else:
                    nc.vector.tensor_tensor(s_l[:], dep[:], iota[:], _ADD)
                    nc.vector.tensor_tensor(vcat[:, 0, :], s_l[:], iota[:], _SUB)
                if og >= 4:
                    nc.gpsimd.tensor_tensor(s_r[:], iota[:], dep[:], _SUB)
                    nc.gpsimd.tensor_tensor(vcat[:, 1, :], iota[:], s_r[:], _SUB)
                else:
                    nc.vector.tensor_tensor(s_r[:], iota[:], dep[:], _SUB)
                    nc.vector.tensor_tensor(vcat[:, 1, :], iota[:], s_r[:], _SUB)

                root = r_pool.tile([HS, 2, 4, WP], F16, tag="root", name="root")
                tree = cfg["tree"]
                ptiles = {}

                for d in range(NTAP):
                    offl, offr = d, PAD - d
                    if d == 0:
                        ptile = root
                    else:
                        ptile = p_pool.tile([HS, 2, 4, WP], F16, tag="p", name="ptile")
                    ptiles[d] = ptile
                    vstep = 4 * WP + (offr - offl)

                    # Zero the margin columns (per view, <=3-dim APs for the
                    # BIR verifier): view-l head [0,d) + tail [W+d, WP);
                    # view-r head [0, 8-d) + tail [W+8-d, WP).
                    mset = nc.gpsimd.memset if cfg["msets_gp"] else nc.vector.memset
                    if d > 0:
                        mset(_ap(ptile[:], 0, [[WP, 4], [1, d]]), 0.0)
                        mset(_ap(ptile[:], 4 * WP + W + PAD - d, [[WP, 4], [1, d]]), 0.0)
                    if d < PAD:
                        mset(_ap(ptile[:], W + d, [[WP, 4], [1, PAD - d]]), 0.0)
                        mset(_ap(ptile[:], 4 * WP, [[WP, 4], [1, PAD - d]]), 0.0)

                    # stage 1: t = |1024*v - 1024*d| for both views (f32).
                    tnow = w_pool.tile([HS, 2, W], F32, tag="t", name="tnow")
                    nc.scalar.activation(tnow[:], vcat[:], _ABS, bias=negd[:, d : d + 1], scale=SCALE)
                    # stage 2 view l (+w'): relu(1024 - t) -> fp16, Act.
                    outl = _ap(ptile[:], 3 * WP + offl, [[1, W]])
                    nc.scalar.activation(outl, tnow[:, 0, :], _RELU, bias=b1024[:], scale=-1.0)
                    # stage 2 view r (-w'): (t-1024) min 0 -> fp16, DVE 2x.
                    outr = _ap(ptile[:], 4 * WP + 3 * WP + offr, [[1, W]])
                    nc.vector.tensor_scalar(outr, tnow[:, 1, :], SCALE, 0.0, op0=_SUB, op1=_MIN)

                    # products for ch 0..2 of both views (fp16); DVE takes
                    # cols [0, W-pg), GP the rest (per view, 3-dim APs).
                    pg = cfg["prod_gp_cols"] if d < cfg["prod_gp_taps"] else 0
                    ps = W - pg
                    if ps > 0:
                        pout = _ap(ptile[:], offl, [[vstep, 2], [WP, 3], [1, ps]])
                        pw = _ap(ptile[:], 3 * WP + offl, [[vstep, 2], [0, 3], [1, ps]])
                        pimg = _ap(img[:], 0, [[0, 2], [W, 3], [1, ps]])
                        nc.vector.tensor_tensor(pout, pw, pimg, _MULT)
                    if pg > 0:
                        for voff in (offl, 4 * WP + offr):
                            po = _ap(ptile[:], voff + ps, [[WP, 3], [1, pg]])
                            pwv = _ap(ptile[:], 3 * WP + voff + ps, [[0, 3], [1, pg]])
                            pim = _ap(img[:], ps, [[W, 3], [1, pg]])
                            nc.gpsimd.tensor_tensor(po, pwv, pim, _MULT)

                    # pair-adds as soon as both taps of a pair exist.
                    if d >= 2 and d % 2 == 0:
                        k = d // 2 - 1  # pair index 0..3
                        full_add(ptiles[d - 1], ptile, tree[k])

                # chain the pair-sums into the root.
                for k in range(4):
                    full_add(root, ptiles[2 * k + 1], tree[4 + k])
                acc = root

                # normalize: rc = 1/(cnt +- eps) in f32, copy to fp16 on Act,
                # then out = acc * rc (fp16, 2x).
                c32 = n_pool.tile([HS, 2, WP], F32, tag="c32", name="c32", bufs=1)
                nc.scalar.activation(c32[:, 0, :], _ap(acc[:], 3 * WP, [[1, WP]]), _IDENT, bias=epsp[:], scale=1.0)
                nc.scalar.activation(c32[:, 1, :], _ap(acc[:], 4 * WP + 3 * WP, [[1, WP]]), _IDENT, bias=epsn[:], scale=1.0)
                rc = n_pool.tile([HS, 2, WP], F32, tag="rc", name="rc", bufs=1)
                nc.vector.reciprocal_approx_fast(out=rc[:], in_=c32[:])
                rc16 = rc

                outst = o_pool.tile([HS, 2, C, W], F16, tag="outst", name="outst")
                mg = cfg["mul_gp_cols"]
                sp = W - mg
                # view-l window starts at col 0, view-r at col PAD.
                macc = _ap(acc[:], 0, [[4 * WP + PAD, 2], [WP, 3], [1, sp]])
                mrc = _ap(rc16[:], 0, [[WP + PAD, 2], [0, 3], [1, sp]])
                mout = _ap(outst[:], 0, [[C * W, 2], [W, 3], [1, sp]])
                nc.vector.tensor_tensor(mout, macc, mrc, _MULT)
                if mg > 0:
                    for vv in range(2):
                        gacc = _ap(acc[:], vv * (4 * WP + PAD) + sp, [[WP, 3], [1, mg]])
                        grc = _ap(rc16[:], vv * (WP + PAD) + sp, [[0, 3], [1, mg]])
                        gout = _ap(outst[:], vv * C * W + sp, [[W, 3], [1, mg]])
                        nc.gpsimd.tensor_tensor(gout, gacc, grc, _MULT)

                if cfg["cast_out"]:
                    nc.gpsimd.dma_start(out=left[b].transpose([1, 0, 2]), in_=outst[:, 0, :, :])
                    nc.gpsimd.dma_start(out=right[b].transpose([1, 0, 2]), in_=outst[:, 1, :, :])
                else:
                    out32 = o_pool.tile([HS, 2, C, W], F32, tag="out32", name="out32")
                    nc.vector.tensor_copy(out32[:], outst[:])
                    nc.sync.dma_start(out=left[b].transpose([1, 0, 2]), in_=out32[:, 0, :, :])
                    nc.sync.dma_start(out=right[b].transpose([1, 0, 2]), in_=out32[:, 1, :, :])
    nc.compile()
    return nc


_NC_CACHE = None


def _get_program():
    global _NC_CACHE
    if _NC_CACHE is None:
        _NC_CACHE = build_program()
    return _NC_CACHE


def kernel(image: np.ndarray, depth: np.ndarray):
    image = np.ascontiguousarray(image, dtype=np.float32)
    depth = np.ascontiguousarray(depth, dtype=np.float32)
    assert image.shape == (B, C, H, W) and depth.shape == (B, H, W)

    nc = _get_program()
    in_maps = []
    for m in range(NCORES):
        sl = slice(m * HS, (m + 1) * HS)
        in_maps.append(
            {
                "image": np.ascontiguousarray(image[:, :, sl, :]),
                "depth": np.ascontiguousarray(depth[:, sl, :]),
            }
        )
    # The axon-tunneled devices occasionally come up in a transient
    # unrecoverable/desynced state and recover on the next attempt —
    # retry once before giving up.
    try:
        res = run_bass_kernel_spmd(nc, in_maps, core_ids=list(range(NCORES)))
    except Exception:
        import time as _time

        _time.sleep(5.0)
        res = run_bass_kernel_spmd(nc, in_maps, core_ids=list(range(NCORES)))
    left = np.concatenate([r["left"] for r in res.results], axis=2)
    right = np.concatenate([r["right"] for r in res.results], axis=2)
    return left, right


# revision 22
# speedup vs baseline: 1.2812x; 1.0259x over previous
"""Dual-pixel depth-merge (forward splat) kernel for Trainium2, 8 NeuronCores.

Math: for integer pixel grid x, the reference computes pos = fl(x +- depth)
(f32-rounded), x0 = floor(pos), f = pos - x0. Define the per-view fractional
offsets
    v_l[i] = fl(i + depth[i]) - i   (exact f32 subtraction, in [0, 8])
    v_r[i] = i - fl(i - depth[i])   (exact f32 subtraction, in [0, 8])
Each view's splat is a 9-tap shifted weighted sum with hat weights
    W_d = relu(1 - |v - d|), d = 0..8.

The bulk of the pipeline runs in fp16 with weights scaled by S=1024:
w' = relu(1024 - |1024 v - 1024 d|). The scaling keeps every nonzero f32
weight (>= 2^-24) a NORMAL fp16 value (>= 2^-14), so hole positions
(count == 0) match the reference bit-exactly and near-hole pixels stay
relatively accurate; the scale cancels in acc/count.

Layout: per tap a padded tile P_d [128, 2(view), 4(rgb+count), 1032]; view l
data sits at column offset +d, view r at 8-d (its output is read at +8), and
the unused margin columns are zeroed, so every accumulation is a plain
aligned full-tile add. Taps accumulate in TWO independent chains (even taps
into acc_e, odd into acc_o) merged at the end — halving the serial-add
critical path. Each add can be column-split across three resources: DVE
tensor_tensor (fp16 2x), GP tensor_tensor, and gpsimd accumulate-DMA
(CCE add on the DMA engines). View r accumulates NEGATED weights (DVE's
(t-1024) min 0 tensor_scalar form, 2x); the sign cancels in acc/count.

Engine roles: Act = weight stage 1 + view-l stage 2 + eps/rc16; DVE =
view-r stage 2, most products, add slices, reciprocal, final mul; GP = add
slices, margin memsets, SWDGE descriptor gen; DMA = i/o + CCE add slices.
Sharding: pure data parallel over h (the scatter is along w only) — core m
takes h rows [m*128, (m+1)*128). No halo, no communication.
"""

import numpy as np

import concourse.bacc as bacc
import concourse.bass as bass
import concourse.mybir as mybir
import concourse.tile as tile
from concourse.bass_utils import run_bass_kernel_spmd

B, C, H, W = 4, 3, 1024, 1024
NCORES = 8
HS = H // NCORES  # 128 h-rows per core
NTAP = 9
PAD = 8
WP = W + PAD  # padded accumulator width
F16 = mybir.dt.float16
F32 = mybir.dt.float32
SCALE = 1024.0
EPS = 1e-10

_ADD = mybir.AluOpType.add
_SUB = mybir.AluOpType.subtract
_MULT = mybir.AluOpType.mult
_MIN = mybir.AluOpType.min
_BYPASS = mybir.AluOpType.bypass
_RELU = mybir.ActivationFunctionType.Relu
_ABS = mybir.ActivationFunctionType.Abs
_IDENT = mybir.ActivationFunctionType.Identity


def _ap(tile_ap, extra_off, dims):
    """Build an AP on tile_ap's tensor: offset += extra_off (elements),
    dims = [[step, num], ...] appended after the partition dim."""
    a = tile_ap.ap
    return bass.AP(
        tensor=tile_ap.tensor,
        offset=tile_ap.offset + extra_off,
        ap=[list(a[0])] + [list(d) for d in dims],
    )


# The 8 accumulates: E-chain adds taps 2,4,6,8 into acc_e; O-chain adds taps
# 3,5,7 into acc_o; "mg" is the final acc_e += acc_o merge. Each value is a
# column split [dve_cols, gp_cols, dma_cols] summing to WP=1032.
CFG = {
    # Accumulation: 4 independent pair-adds (P1+=P2, P3+=P4, P5+=P6,
    # P7+=P8) then a chain into the root tap-0 tile (R+=S1..S4). Engine per
    # add: 'v' DVE, 'g' GP STT, 'm' gpsimd accumulate-DMA.
    # CCE accumulate-DMA ('m') wedges the real device (NRT_EXEC_UNIT_
    # UNRECOVERABLE) even though it models well, so adds stay on DVE/GP.
    "tree": ["v", {"v": 720, "g": 312}, {"v": 720, "g": 312}, "v",
             "v", {"v": 720, "g": 312}, {"v": 720, "g": 312}, "v"],
    "prod_gp_cols": 128,   # product columns (per view) handled by GP
    "prod_gp_taps": 5,   # taps (d < this) eligible for the GP product split
    "off_gp": 2,         # offset ops on GP (0, 2 or 4)
    "mul_gp_cols": 384,    # final-mul columns (per view) handled by GP
    "msets_gp": True,    # margin memsets on GP (else DVE)
    "cce_single_packet": True,
    "cast_in": True,     # load image via casting gpsimd DMA
    "cast_out": True,    # store outputs via casting gpsimd DMA
    "io_bufs": 2,
    "p_bufs": 5,
    "acc_bufs": 2,
    "out_bufs": 1,
}


def build_program(cfg: dict | None = None) -> bass.Bass:
    cfg = {**CFG, **(cfg or {})}
    nc = bacc.Bacc()
    image = nc.dram_tensor("image", [B, C, HS, W], F32, kind="ExternalInput")
    depth = nc.dram_tensor("depth", [B, HS, W], F32, kind="ExternalInput")
    left = nc.dram_tensor("left", [B, C, HS, W], F32, kind="ExternalOutput")
    right = nc.dram_tensor("right", [B, C, HS, W], F32, kind="ExternalOutput")

    with tile.TileContext(nc) as tc:
        with (
            tc.tile_pool(name="consts", bufs=1) as c_pool,
            tc.tile_pool(name="io", bufs=cfg["io_bufs"]) as io_pool,
            tc.tile_pool(name="voff", bufs=2) as v_pool,
            tc.tile_pool(name="wts", bufs=2) as w_pool,
            tc.tile_pool(name="prod", bufs=cfg["p_bufs"]) as p_pool,
            tc.tile_pool(name="root", bufs=cfg["acc_bufs"]) as r_pool,
            tc.tile_pool(name="norm", bufs=2) as n_pool,
            tc.tile_pool(name="outs", bufs=cfg["out_bufs"]) as o_pool,
        ):
            iota = c_pool.tile([HS, W], F32, tag="iota", name="iota")
            iota_iv = iota[:].bitcast(mybir.dt.int32)
            nc.gpsimd.iota(iota_iv, [[1, W]], channel_multiplier=0)
            nc.vector.tensor_copy(iota[:], iota_iv)
            epsp = c_pool.tile([HS, 1], F32, tag="epsp", name="epsp")
            nc.vector.memset(epsp[:], EPS)
            epsn = c_pool.tile([HS, 1], F32, tag="epsn", name="epsn")
            nc.vector.memset(epsn[:], -EPS)
            # Per-tap stage-1 biases (-1024*d) and the stage-2 bias (+1024).
            negd = c_pool.tile([HS, NTAP], F32, tag="negd", name="negd")
            for d in range(NTAP):
                nc.vector.memset(negd[:, d : d + 1], -SCALE * d)
            b1024 = c_pool.tile([HS, 1], F32, tag="b1024", name="b1024")
            nc.vector.memset(b1024[:], SCALE)

            def full_add(dst, src, spec):
                """dst += src over the whole [HS, 2, 4, WP] tile. spec is an
                engine char ('v' DVE, 'g' GP, 'm' accumulate-DMA) or a dict
                {eng: cols} column-splitting the add across engines."""
                if isinstance(spec, str):
                    spec = {spec: WP}
                a = 0
                for eng, wid in spec.items():
                    if wid == 0:
                        continue
                    daps = _ap(dst[:], a, [[WP, 8], [1, wid]])
                    saps = _ap(src[:], a, [[WP, 8], [1, wid]])
                    if eng == "v":
                        nc.vector.tensor_tensor(daps, daps, saps, _ADD)
                    elif eng == "g":
                        nc.gpsimd.tensor_tensor(daps, daps, saps, _ADD)
                    else:
                        nc.gpsimd.dma_start(out=daps, in_=saps, accum_op=_ADD, single_packet=cfg["cce_single_packet"])
                    a += wid
                    assert a <= WP


            for b in range(B):
                dep = io_pool.tile([HS, W], F32, tag="dep", name="dep")
                nc.sync.dma_start(out=dep[:], in_=depth[b])
                if cfg["cast_in"]:
                    img = io_pool.tile([HS, C, W], F16, tag="img", name="img")
                    nc.gpsimd.dma_start(out=img[:], in_=image[b].transpose([1, 0, 2]))
                else:
                    img32 = io_pool.tile([HS, C, W], F32, tag="img32", name="img32")
                    nc.sync.dma_start(out=img32[:], in_=image[b].transpose([1, 0, 2]))
                    img = io_pool.tile([HS, C, W], F16, tag="img", name="img")
                    nc.vector.tensor_copy(img[:], img32[:])

                # v offsets, f32-exact. vcat[:,0,:] = vl, vcat[:,1,:] = vr.
                vcat = v_pool.tile([HS, 2, W], F32, tag="vcat", name="vcat")
                s_l = v_pool.tile([HS, W], F32, tag="sl", name="s_l", bufs=1)
                s_r = v_pool.tile([HS, W], F32, tag="sr", name="s_r", bufs=1)
                og = cfg["off_gp"]
                if og >= 2:
                    nc.gpsimd.tensor_tensor(s_l[:], dep[:], iota[:], _ADD)
                    nc.gpsimd.tensor_tensor(vcat[:, 0, :], s_l[:], iota[:], _SUB)
                else:
                    nc.vector.tensor_tensor(s_l[:], dep[:], iota[:], _ADD)
                    nc.vector.tensor_tensor(vcat[:, 0, :], s_l[:], iota[:], _SUB)
                if og >= 4:
                    nc.gpsimd.tensor_tensor(s_r[:], iota[:], dep[:], _SUB)
                    nc.gpsimd.tensor_tensor(vcat[:, 1, :], iota[:], s_r[:], _SUB)
                else:
                    nc.vector.tensor_tensor(s_r[:], iota[:], dep[:], _SUB)
                    nc.vector.tensor_tensor(vcat[:, 1, :], iota[:], s_r[:], _SUB)

                root = r_pool.tile([HS, 2, 4, WP], F16, tag="root", name="root")
                tree = cfg["tree"]
                ptiles = {}

                for d in range(NTAP):
                    offl, offr = d, PAD - d
                    if d == 0:
                        ptile = root
                    else:
                        ptile = p_pool.tile([HS, 2, 4, WP], F16, tag="p", name="ptile")
                    ptiles[d] = ptile
                    vstep = 4 * WP + (offr - offl)

                    # Zero the margin columns (per view, <=3-dim APs for the
                    # BIR verifier): view-l head [0,d) + tail [W+d, WP);
                    # view-r head [0, 8-d) + tail [W+8-d, WP).
                    mset = nc.gpsimd.memset if cfg["msets_gp"] else nc.vector.memset
                    if d > 0:
                        mset(_ap(ptile[:], 0, [[WP, 4], [1, d]]), 0.0)
                        mset(_ap(ptile[:], 4 * WP + W + PAD - d, [[WP, 4], [1, d]]), 0.0)
                    if d < PAD:
                        mset(_ap(ptile[:], W + d, [[WP, 4], [1, PAD - d]]), 0.0)
                        mset(_ap(ptile[:], 4 * WP, [[WP, 4], [1, PAD - d]]), 0.0)

                    # stage 1: t = |1024*v - 1024*d| for both views (f32).
                    tnow = w_pool.tile([HS, 2, W], F32, tag="t", name="tnow")
                    nc.scalar.activation(tnow[:], vcat[:], _ABS, bias=negd[:, d : d + 1], scale=SCALE)
                    # stage 2 view l (+w'): relu(1024 - t) -> fp16, Act.
                    outl = _ap(ptile[:], 3 * WP + offl, [[1, W]])
                    nc.scalar.activation(outl, tnow[:, 0, :], _RELU, bias=b1024[:], scale=-1.0)
                    # stage 2 view r (-w'): (t-1024) min 0 -> fp16, DVE 2x.
                    outr = _ap(ptile[:], 4 * WP + 3 * WP + offr, [[1, W]])
                    nc.vector.tensor_scalar(outr, tnow[:, 1, :], SCALE, 0.0, op0=_SUB, op1=_MIN)

                    # products for ch 0..2 of both views (fp16); DVE takes
                    # cols [0, W-pg), GP the rest (per view, 3-dim APs).
                    pg = cfg["prod_gp_cols"] if d < cfg["prod_gp_taps"] else 0
                    ps = W - pg
                    if ps > 0:
                        pout = _ap(ptile[:], offl, [[vstep, 2], [WP, 3], [1, ps]])
                        pw = _ap(ptile[:], 3 * WP + offl, [[vstep, 2], [0, 3], [1, ps]])
                        pimg = _ap(img[:], 0, [[0, 2], [W, 3], [1, ps]])
                        nc.vector.tensor_tensor(pout, pw, pimg, _MULT)
                    if pg > 0:
                        for voff in (offl, 4 * WP + offr):
                            po = _ap(ptile[:], voff + ps, [[WP, 3], [1, pg]])
                            pwv = _ap(ptile[:], 3 * WP + voff + ps, [[0, 3], [1, pg]])
                            pim = _ap(img[:], ps, [[W, 3], [1, pg]])
                            nc.gpsimd.tensor_tensor(po, pwv, pim, _MULT)

                    # pair-adds as soon as both taps of a pair exist.
                    if d >= 2 and d % 2 == 0:
                        k = d // 2 - 1  # pair index 0..3
                        full_add(ptiles[d - 1], ptile, tree[k])

                # chain the pair-sums into the root.
                for k in range(4):
                    full_add(root, ptiles[2 * k + 1], tree[4 + k])
                acc = root

                # normalize: rc = 1/(cnt +- eps) in f32, copy to fp16 on Act,
                # then out = acc * rc (fp16, 2x).
                c32 = n_pool.tile([HS, 2, WP], F32, tag="c32", name="c32", bufs=1)
                nc.scalar.activation(c32[:, 0, :], _ap(acc[:], 3 * WP, [[1, WP]]), _IDENT, bias=epsp[:], scale=1.0)
                nc.scalar.activation(c32[:, 1, :], _ap(acc[:], 4 * WP + 3 * WP, [[1, WP]]), _IDENT, bias=epsn[:], scale=1.0)
                rc = n_pool.tile([HS, 2, WP], F32, tag="rc", name="rc", bufs=1)
                nc.vector.reciprocal_approx_fast(out=rc[:], in_=c32[:])
                rc16 = rc

                outst = o_pool.tile([HS, 2, C, W], F16, tag="outst", name="outst")
                mg = cfg["mul_gp_cols"]
                sp = W - mg
                # view-l window starts at col 0, view-r at col PAD.
                macc = _ap(acc[:], 0, [[4 * WP + PAD, 2], [WP, 3], [1, sp]])
                mrc = _ap(rc16[:], 0, [[WP + PAD, 2], [0, 3], [1, sp]])
                mout = _ap(outst[:], 0, [[C * W, 2], [W, 3], [1, sp]])
                nc.vector.tensor_tensor(mout, macc, mrc, _MULT)
                if mg > 0:
                    for vv in range(2):
                        gacc = _ap(acc[:], vv * (4 * WP + PAD) + sp, [[WP, 3], [1, mg]])
                        grc = _ap(rc16[:], vv * (WP + PAD) + sp, [[0, 3], [1, mg]])
                        gout = _ap(outst[:], vv * C * W + sp, [[W, 3], [1, mg]])
                        nc.gpsimd.tensor_tensor(gout, gacc, grc, _MULT)

                if cfg["cast_out"]:
                    nc.gpsimd.dma_start(out=left[b].transpose([1, 0, 2]), in_=outst[:, 0, :, :])
                    nc.gpsimd.dma_start(out=right[b].transpose([1, 0, 2]), in_=outst[:, 1, :, :])
                else:
                    out32 = o_pool.tile([HS, 2, C, W], F32, tag="out32", name="out32")
                    nc.vector.tensor_copy(out32[:], outst[:])
                    nc.sync.dma_start(out=left[b].transpose([1, 0, 2]), in_=out32[:, 0, :, :])
                    nc.sync.dma_start(out=right[b].transpose([1, 0, 2]), in_=out32[:, 1, :, :])
    nc.compile()
    return nc


_NC_CACHE = None


def _get_program():
    global _NC_CACHE
    if _NC_CACHE is None:
        _NC_CACHE = build_program()
    return _NC_CACHE


def kernel(image: np.ndarray, depth: np.ndarray):
    image = np.ascontiguousarray(image, dtype=np.float32)
    depth = np.ascontiguousarray(depth, dtype=np.float32)
    assert image.shape == (B, C, H, W) and depth.shape == (B, H, W)

    nc = _get_program()
    in_maps = []
    for m in range(NCORES):
        sl = slice(m * HS, (m + 1) * HS)
        in_maps.append(
            {
                "image": np.ascontiguousarray(image[:, :, sl, :]),
                "depth": np.ascontiguousarray(depth[:, sl, :]),
            }
        )
    # The axon-tunneled devices occasionally come up in a transient
    # unrecoverable/desynced state and recover on the next attempt —
    # retry once before giving up.
    try:
        res = run_bass_kernel_spmd(nc, in_maps, core_ids=list(range(NCORES)))
    except Exception:
        import time as _time

        _time.sleep(5.0)
        res = run_bass_kernel_spmd(nc, in_maps, core_ids=list(range(NCORES)))
    left = np.concatenate([r["left"] for r in res.results], axis=2)
    right = np.concatenate([r["right"] for r in res.results], axis=2)
    return left, right


# revision 24
# speedup vs baseline: 1.3278x; 1.0363x over previous
"""Dual-pixel depth-merge (forward splat) kernel for Trainium2, 8 NeuronCores.

Math: for integer pixel grid x, the reference computes pos = fl(x +- depth)
(f32-rounded), x0 = floor(pos), f = pos - x0. Define the per-view fractional
offsets
    v_l[i] = fl(i + depth[i]) - i   (exact f32 subtraction, in [0, 8])
    v_r[i] = i - fl(i - depth[i])   (exact f32 subtraction, in [0, 8])
Each view's splat is a 9-tap shifted weighted sum with hat weights
    W_d = relu(1 - |v - d|), d = 0..8.

The bulk of the pipeline runs in fp16 with weights scaled by S=1024:
w' = relu(1024 - |1024 v - 1024 d|). The scaling keeps every nonzero f32
weight (>= 2^-24) a NORMAL fp16 value (>= 2^-14), so hole positions
(count == 0) match the reference bit-exactly and near-hole pixels stay
relatively accurate; the scale cancels in acc/count.

Layout: per tap a padded tile P_d [128, 2(view), 4(rgb+count), 1032]; view l
data sits at column offset +d, view r at 8-d (its output is read at +8), and
the unused margin columns are zeroed, so every accumulation is a plain
aligned full-tile add. Taps accumulate in TWO independent chains (even taps
into acc_e, odd into acc_o) merged at the end — halving the serial-add
critical path. Each add can be column-split across three resources: DVE
tensor_tensor (fp16 2x), GP tensor_tensor, and gpsimd accumulate-DMA
(CCE add on the DMA engines). View r accumulates NEGATED weights (DVE's
(t-1024) min 0 tensor_scalar form, 2x); the sign cancels in acc/count.

Engine roles: Act = weight stage 1 + view-l stage 2 + eps/rc16; DVE =
view-r stage 2, most products, add slices, reciprocal, final mul; GP = add
slices, margin memsets, SWDGE descriptor gen; DMA = i/o + CCE add slices.
Sharding: pure data parallel over h (the scatter is along w only) — core m
takes h rows [m*128, (m+1)*128). No halo, no communication.
"""

import numpy as np

import concourse.bacc as bacc
import concourse.bass as bass
import concourse.mybir as mybir
import concourse.tile as tile
from concourse.bass_utils import run_bass_kernel_spmd

B, C, H, W = 4, 3, 1024, 1024
NCORES = 8
HS = H // NCORES  # 128 h-rows per core
NTAP = 9
PAD = 8
WP = W + PAD  # padded accumulator width
F16 = mybir.dt.float16
F32 = mybir.dt.float32
SCALE = 1024.0
EPS = 1e-10

_ADD = mybir.AluOpType.add
_SUB = mybir.AluOpType.subtract
_MULT = mybir.AluOpType.mult
_MIN = mybir.AluOpType.min
_BYPASS = mybir.AluOpType.bypass
_RELU = mybir.ActivationFunctionType.Relu
_ABS = mybir.ActivationFunctionType.Abs
_IDENT = mybir.ActivationFunctionType.Identity


def _ap(tile_ap, extra_off, dims):
    """Build an AP on tile_ap's tensor: offset += extra_off (elements),
    dims = [[step, num], ...] appended after the partition dim."""
    a = tile_ap.ap
    return bass.AP(
        tensor=tile_ap.tensor,
        offset=tile_ap.offset + extra_off,
        ap=[list(a[0])] + [list(d) for d in dims],
    )


# The 8 accumulates: E-chain adds taps 2,4,6,8 into acc_e; O-chain adds taps
# 3,5,7 into acc_o; "mg" is the final acc_e += acc_o merge. Each value is a
# column split [dve_cols, gp_cols, dma_cols] summing to WP=1032.
CFG = {
    # Accumulation: 4 independent pair-adds (P1+=P2, P3+=P4, P5+=P6,
    # P7+=P8) then a chain into the root tap-0 tile (R+=S1..S4). Engine per
    # add: 'v' DVE, 'g' GP STT, 'm' gpsimd accumulate-DMA.
    # CCE accumulate-DMA ('m') wedges the real device (NRT_EXEC_UNIT_
    # UNRECOVERABLE) even though it models well, so adds stay on DVE/GP.
    "tree": ["v", {"v": 720, "g": 312}, {"v": 720, "g": 312}, "v",
             "v", {"v": 720, "g": 312}, {"v": 720, "g": 312}, "v"],
    "prod_gp_cols": 128,   # product columns (per view) handled by GP
    "prod_gp_taps": 5,   # taps (d < this) eligible for the GP product split
    "off_gp": 2,         # offset ops on GP (0, 2 or 4)
    "mul_gp_cols": 384,    # final-mul columns (per view) handled by GP
    "msets_gp": True,    # margin memsets on GP (else DVE)
    "cce_single_packet": True,
    "cast_in": True,     # load image via casting gpsimd DMA
    "cast_out": True,    # store outputs via casting gpsimd DMA
    "io_bufs": 2,
    "p_bufs": 5,
    "acc_bufs": 2,
    "out_bufs": 1,
}


def build_program(cfg: dict | None = None) -> bass.Bass:
    cfg = {**CFG, **(cfg or {})}
    nc = bacc.Bacc()
    image = nc.dram_tensor("image", [B, C, HS, W], F32, kind="ExternalInput")
    depth = nc.dram_tensor("depth", [B, HS, W], F32, kind="ExternalInput")
    left = nc.dram_tensor("left", [B, C, HS, W], F32, kind="ExternalOutput")
    right = nc.dram_tensor("right", [B, C, HS, W], F32, kind="ExternalOutput")

    with tile.TileContext(nc) as tc:
        with (
            tc.tile_pool(name="consts", bufs=1) as c_pool,
            tc.tile_pool(name="io", bufs=cfg["io_bufs"]) as io_pool,
            tc.tile_pool(name="voff", bufs=2) as v_pool,
            tc.tile_pool(name="wts", bufs=2) as w_pool,
            tc.tile_pool(name="prod", bufs=cfg["p_bufs"]) as p_pool,
            tc.tile_pool(name="root", bufs=cfg["acc_bufs"]) as r_pool,
            tc.tile_pool(name="norm", bufs=2) as n_pool,
            tc.tile_pool(name="outs", bufs=cfg["out_bufs"]) as o_pool,
        ):
            iota = c_pool.tile([HS, W], F32, tag="iota", name="iota")
            iota_iv = iota[:].bitcast(mybir.dt.int32)
            nc.gpsimd.iota(iota_iv, [[1, W]], channel_multiplier=0)
            nc.vector.tensor_copy(iota[:], iota_iv)
            epsp = c_pool.tile([HS, 1], F32, tag="epsp", name="epsp")
            nc.vector.memset(epsp[:], EPS)
            epsn = c_pool.tile([HS, 1], F32, tag="epsn", name="epsn")
            nc.vector.memset(epsn[:], -EPS)
            # Per-tap stage-1 biases (-1024*d) and the stage-2 bias (+1024).
            negd = c_pool.tile([HS, NTAP], F32, tag="negd", name="negd")
            for d in range(NTAP):
                nc.vector.memset(negd[:, d : d + 1], -SCALE * d)
            b1024 = c_pool.tile([HS, 1], F32, tag="b1024", name="b1024")
            nc.vector.memset(b1024[:], SCALE)

            def full_add(dst, src, spec):
                """dst += src over the whole [HS, 2, 4, WP] tile. spec is an
                engine char ('v' DVE, 'g' GP, 'm' accumulate-DMA) or a dict
                {eng: cols} column-splitting the add across engines."""
                if isinstance(spec, str):
                    spec = {spec: WP}
                a = 0
                for eng, wid in spec.items():
                    if wid == 0:
                        continue
                    daps = _ap(dst[:], a, [[WP, 8], [1, wid]])
                    saps = _ap(src[:], a, [[WP, 8], [1, wid]])
                    if eng == "v":
                        nc.vector.tensor_tensor(daps, daps, saps, _ADD)
                    elif eng == "g":
                        nc.gpsimd.tensor_tensor(daps, daps, saps, _ADD)
                    else:
                        nc.gpsimd.dma_start(out=daps, in_=saps, accum_op=_ADD, single_packet=cfg["cce_single_packet"])
                    a += wid
                    assert a <= WP


            for b in range(B):
                dep = io_pool.tile([HS, W], F32, tag="dep", name="dep")
                nc.sync.dma_start(out=dep[:], in_=depth[b])
                if cfg["cast_in"]:
                    img = io_pool.tile([HS, C, W], F16, tag="img", name="img")
                    nc.gpsimd.dma_start(out=img[:], in_=image[b].transpose([1, 0, 2]))
                else:
                    img32 = io_pool.tile([HS, C, W], F32, tag="img32", name="img32")
                    nc.sync.dma_start(out=img32[:], in_=image[b].transpose([1, 0, 2]))
                    img = io_pool.tile([HS, C, W], F16, tag="img", name="img")
                    nc.vector.tensor_copy(img[:], img32[:])

                # v offsets, f32-exact. vcat[:,0,:] = vl, vcat[:,1,:] = vr.
                vcat = v_pool.tile([HS, 2, W], F32, tag="vcat", name="vcat")
                s_l = v_pool.tile([HS, W], F32, tag="sl", name="s_l", bufs=1)
                s_r = v_pool.tile([HS, W], F32, tag="sr", name="s_r", bufs=1)
                og = cfg["off_gp"]
                if og >= 2:
                    nc.gpsimd.tensor_tensor(s_l[:], dep[:], iota[:], _ADD)
                    nc.gpsimd.tensor_tensor(vcat[:, 0, :], s_l[:], iota[:], _SUB)
                else:
                    nc.vector.tensor_tensor(s_l[:], dep[:], iota[:], _ADD)
                    nc.vector.tensor_tensor(vcat[:, 0, :], s_l[:], iota[:], _SUB)
                if og >= 4:
                    nc.gpsimd.tensor_tensor(s_r[:], iota[:], dep[:], _SUB)
                    nc.gpsimd.tensor_tensor(vcat[:, 1, :], iota[:], s_r[:], _SUB)
                else:
                    nc.vector.tensor_tensor(s_r[:], iota[:], dep[:], _SUB)
                    nc.vector.tensor_tensor(vcat[:, 1, :], iota[:], s_r[:], _SUB)

                root = r_pool.tile([HS, 2, 4, WP], F16, tag="root", name="root")
                tree = cfg["tree"]
                ptiles = {}

                for d in range(NTAP):
                    offl, offr = d, PAD - d
                    if d == 0:
                        ptile = root
                    else:
                        ptile = p_pool.tile([HS, 2, 4, WP], F16, tag="p", name="ptile")
                    ptiles[d] = ptile
                    vstep = 4 * WP + (offr - offl)

                    # Zero the margin columns (per view, <=3-dim APs for the
                    # BIR verifier): view-l head [0,d) + tail [W+d, WP);
                    # view-r head [0, 8-d) + tail [W+8-d, WP).
                    mset = nc.gpsimd.memset if cfg["msets_gp"] else nc.vector.memset
                    if d > 0:
                        mset(_ap(ptile[:], 0, [[WP, 4], [1, d]]), 0.0)
                        mset(_ap(ptile[:], 4 * WP + W + PAD - d, [[WP, 4], [1, d]]), 0.0)
                    if d < PAD:
                        mset(_ap(ptile[:], W + d, [[WP, 4], [1, PAD - d]]), 0.0)
                        mset(_ap(ptile[:], 4 * WP, [[WP, 4], [1, PAD - d]]), 0.0)

                    # stage 1: t = |1024*v - 1024*d| for both views (f32).
                    tnow = w_pool.tile([HS, 2, W], F32, tag="t", name="tnow")
                    nc.scalar.activation(tnow[:], vcat[:], _ABS, bias=negd[:, d : d + 1], scale=SCALE)
                    # stage 2 both views (+w'): relu(1024 - t) -> fp16 in one
                    # Act op (view-step windowed output); DVE is the bottleneck
                    # so the scalar engine takes both views.
                    outw = _ap(ptile[:], 3 * WP + offl, [[vstep, 2], [1, W]])
                    nc.scalar.activation(outw, tnow[:], _RELU, bias=b1024[:], scale=-1.0)

                    # products for ch 0..2 of both views (fp16); DVE takes
                    # cols [0, W-pg), GP the rest (per view, 3-dim APs).
                    pg = cfg["prod_gp_cols"] if d < cfg["prod_gp_taps"] else 0
                    ps = W - pg
                    if ps > 0:
                        pout = _ap(ptile[:], offl, [[vstep, 2], [WP, 3], [1, ps]])
                        pw = _ap(ptile[:], 3 * WP + offl, [[vstep, 2], [0, 3], [1, ps]])
                        pimg = _ap(img[:], 0, [[0, 2], [W, 3], [1, ps]])
                        nc.vector.tensor_tensor(pout, pw, pimg, _MULT)
                    if pg > 0:
                        for voff in (offl, 4 * WP + offr):
                            po = _ap(ptile[:], voff + ps, [[WP, 3], [1, pg]])
                            pwv = _ap(ptile[:], 3 * WP + voff + ps, [[0, 3], [1, pg]])
                            pim = _ap(img[:], ps, [[W, 3], [1, pg]])
                            nc.gpsimd.tensor_tensor(po, pwv, pim, _MULT)

                    # pair-adds as soon as both taps of a pair exist.
                    if d >= 2 and d % 2 == 0:
                        k = d // 2 - 1  # pair index 0..3
                        full_add(ptiles[d - 1], ptile, tree[k])

                # chain the pair-sums into the root.
                for k in range(4):
                    full_add(root, ptiles[2 * k + 1], tree[4 + k])
                acc = root

                # normalize: rc = 1/(cnt +- eps) in f32, copy to fp16 on Act,
                # then out = acc * rc (fp16, 2x).
                c32 = n_pool.tile([HS, 2, WP], F32, tag="c32", name="c32", bufs=1)
                nc.scalar.activation(c32[:, 0, :], _ap(acc[:], 3 * WP, [[1, WP]]), _IDENT, bias=epsp[:], scale=1.0)
                nc.scalar.activation(c32[:, 1, :], _ap(acc[:], 4 * WP + 3 * WP, [[1, WP]]), _IDENT, bias=epsp[:], scale=1.0)
                rc = n_pool.tile([HS, 2, WP], F32, tag="rc", name="rc", bufs=1)
                nc.vector.reciprocal_approx_fast(out=rc[:], in_=c32[:])
                rc16 = rc

                outst = o_pool.tile([HS, 2, C, W], F16, tag="outst", name="outst")
                mg = cfg["mul_gp_cols"]
                sp = W - mg
                # view-l window starts at col 0, view-r at col PAD.
                macc = _ap(acc[:], 0, [[4 * WP + PAD, 2], [WP, 3], [1, sp]])
                mrc = _ap(rc16[:], 0, [[WP + PAD, 2], [0, 3], [1, sp]])
                mout = _ap(outst[:], 0, [[C * W, 2], [W, 3], [1, sp]])
                nc.vector.tensor_tensor(mout, macc, mrc, _MULT)
                if mg > 0:
                    for vv in range(2):
                        gacc = _ap(acc[:], vv * (4 * WP + PAD) + sp, [[WP, 3], [1, mg]])
                        grc = _ap(rc16[:], vv * (WP + PAD) + sp, [[0, 3], [1, mg]])
                        gout = _ap(outst[:], vv * C * W + sp, [[W, 3], [1, mg]])
                        nc.gpsimd.tensor_tensor(gout, gacc, grc, _MULT)

                if cfg["cast_out"]:
                    nc.gpsimd.dma_start(out=left[b].transpose([1, 0, 2]), in_=outst[:, 0, :, :])
                    nc.gpsimd.dma_start(out=right[b].transpose([1, 0, 2]), in_=outst[:, 1, :, :])
                else:
                    out32 = o_pool.tile([HS, 2, C, W], F32, tag="out32", name="out32")
                    nc.vector.tensor_copy(out32[:], outst[:])
                    nc.sync.dma_start(out=left[b].transpose([1, 0, 2]), in_=out32[:, 0, :, :])
                    nc.sync.dma_start(out=right[b].transpose([1, 0, 2]), in_=out32[:, 1, :, :])
    nc.compile()
    return nc


_NC_CACHE = None


def _get_program():
    global _NC_CACHE
    if _NC_CACHE is None:
        _NC_CACHE = build_program()
    return _NC_CACHE


def kernel(image: np.ndarray, depth: np.ndarray):
    image = np.ascontiguousarray(image, dtype=np.float32)
    depth = np.ascontiguousarray(depth, dtype=np.float32)
    assert image.shape == (B, C, H, W) and depth.shape == (B, H, W)

    nc = _get_program()
    in_maps = []
    for m in range(NCORES):
        sl = slice(m * HS, (m + 1) * HS)
        in_maps.append(
            {
                "image": np.ascontiguousarray(image[:, :, sl, :]),
                "depth": np.ascontiguousarray(depth[:, sl, :]),
            }
        )
    # The axon-tunneled devices occasionally come up in a transient
    # unrecoverable/desynced state and recover on the next attempt —
    # retry once before giving up.
    try:
        res = run_bass_kernel_spmd(nc, in_maps, core_ids=list(range(NCORES)))
    except Exception:
        import time as _time

        _time.sleep(5.0)
        res = run_bass_kernel_spmd(nc, in_maps, core_ids=list(range(NCORES)))
    left = np.concatenate([r["left"] for r in res.results], axis=2)
    right = np.concatenate([r["right"] for r in res.results], axis=2)
    return left, right
